# revision 58
# baseline (speedup 1.0000x reference)
"""AKConv TRN2 kernel: 8-core data-parallel over batch.

Sample-major gather architecture: dma_gather(transpose=False) puts each
sample (4 corners x 128ch) on one partition; bilinear combine on DVE via
stride-0 broadcast weights; PE transposes chunks back to channel-major,
accumulates the BN gram + row sums off the same stationary, and runs the
1x1 conv for both outc halves inline. Tail: allreduce -> BN coeffs ->
SiLU -> output DMA.
"""
import sys
sys.path.insert(0, "/opt/trn_rl_repo")
import math
import numpy as np
import ml_dtypes
import bass_rust
import concourse.bass as bass
import concourse.tile as tile
from concourse import bacc, mybir
from concourse.bass_utils import run_bass_kernel_spmd

F32 = mybir.dt.float32
BF16 = mybir.dt.bfloat16
I16 = mybir.dt.int16
AF = mybir.ActivationFunctionType
OP = mybir.AluOpType

B, INC, H, W = 8, 128, 64, 64
OUTC, N = 256, 9
S = H * W                      # 4096 pixels per core
NS = N * S                     # 36864 samples per core
S_TOT = float(B * NS)          # BN sample count
EPS = 1e-5
N_CORES = 8

GH = 2048                      # samples per dma_gather
NG = NS // GH                  # 18 gathers
NCHUNK = GH // 128             # 16 chunks of 128 samples per gather
SPLIT_CHUNK = 256              # close gram A after pair 7 (8/9 of chunks)


def _ap_raw(ap, offset, dims):
    a = ap.copy()
    a.offset = offset
    a.ap = bass_rust.VecI64Pair(dims)
    return a


def build(stage=3, new_idx=True, new_w=True):
    nc = bacc.Bacc("TRN2", target_bir_lowering=False, debug=False,
                   num_devices=N_CORES)
    xpad_d = nc.dram_tensor("xpad", [INC, 66 * 66], BF16, kind="ExternalInput")
    x2_d = nc.dram_tensor("x2", [S, 2 * INC], BF16, kind="ExternalInput")
    pwt_d = nc.dram_tensor("pwt", [INC, 9, 2 * N], F32, kind="ExternalInput")
    base4_d = nc.dram_tensor("base4", [128, 32, 2 * N], F32, kind="ExternalInput")
    cwt_d = nc.dram_tensor("cwt", [INC, OUTC], F32, kind="ExternalInput")
    gb_d = nc.dram_tensor("gb", [1, 2 * OUTC], F32, kind="ExternalInput")
    id18_d = nc.dram_tensor("id18", [18, 18], F32, kind="ExternalInput")
    id128_d = nc.dram_tensor("id128", [128, 128], BF16, kind="ExternalInput")
    id128f_d = nc.dram_tensor("id128f", [128, 128], F32, kind="ExternalInput")
    out_d = nc.dram_tensor("out", [OUTC, NS], BF16, kind="ExternalOutput")

    idx_dram = nc.dram_tensor("idx_scratch", [1, NS], F32, kind="Internal")
    w_dram = nc.dram_tensor("w_scratch", [4, NS], BF16, kind="Internal")
    ab_dram = nc.dram_tensor("ab_scratch", [OUTC, 2], F32, kind="Internal")
    y1_dram = nc.dram_tensor("y1_scratch", [128, NS], BF16, kind="Internal")

    with tile.TileContext(nc) as tc:
        with tc.tile_pool(name="persist", bufs=1) as pp, \
             tc.tile_pool(name="work", bufs=1) as wp, \
             tc.tile_pool(name="dram", bufs=1, space="DRAM") as dp:

            # ---------- loads (spread across HWDGE rings, xpad first) ----------
            warm_in = dp.tile([1, 4], F32)
            warm_out = dp.tile([1, 4], F32, addr_space="Shared")
            if stage >= 3:
                nc.sync.dma_start(warm_in[:], gb_d[:, 0:4])
                nc.gpsimd.collective_compute(
                    "AllReduce", OP.add,
                    replica_groups=[list(range(N_CORES))],
                    ins=[warm_in[:].opt()],
                    outs=[warm_out[:].opt()])
            _egp_cm = tc.tile_pool(name="earlygp", bufs=2)
            egp = _egp_cm.__enter__()
            _xp_cm = tc.tile_pool(name="xpadp", bufs=1)
            xp = _xp_cm.__enter__()
            xpad = xp.tile([INC, 66 * 66], BF16)
            nc.sync.dma_start(xpad[:], xpad_d[:])
            pwt = pp.tile([INC, 9, 2 * N], F32)
            nc.scalar.dma_start(pwt[:], pwt_d[:])
            pwt_b = pp.tile([INC, 9, 2 * N], BF16)
            nc.vector.tensor_copy(pwt_b[:], pwt[:])
            id18 = pp.tile([18, 18], F32)
            nc.scalar.dma_start(id18[:], id18_d[:])
            id128 = pp.tile([128, 128], BF16)
            nc.scalar.dma_start(id128[:], id128_d[:])
            id128f = pp.tile([128, 128], F32)
            nc.scalar.dma_start(id128f[:], id128f_d[:])
            base4 = pp.tile([128, 32, 2 * N], F32)
            nc.scalar.dma_start(base4[:], base4_d[:])
            cwt = pp.tile([INC, OUTC], F32)
            nc.scalar.dma_start(cwt[:], cwt_d[:])
            gb = pp.tile([1, 2 * OUTC], F32)
            nc.scalar.dma_start(gb[:], gb_d[:])
            cwt_b = pp.tile([INC, OUTC], BF16)
            nc.vector.tensor_copy(cwt_b[:], cwt[:])
            ones_b = pp.tile([128, 1], BF16)
            nc.vector.memset(ones_b[:], 1.0)
            ones_f = pp.tile([128, 1], F32)
            nc.vector.memset(ones_f[:], 1.0)
            posT = pp.tile([128, 32, 2 * N], F32)

            # ---------- sliced prologue: produce t-slabs [t0, t1) ----------
            def ts(out, in_, s1, o1, s2=None, o2=None):
                if s2 is None:
                    nc.vector.tensor_scalar(out, in_, s1, None, op0=o1)
                else:
                    nc.vector.tensor_scalar(out, in_, s1, s2, op0=o1, op1=o2)

            _pcp_cm = tc.tile_pool(name="psum_pc", bufs=2, space="PSUM")
            pcp = _pcp_cm.__enter__()
            _pwr_cm = tc.tile_pool(name="psum_wr", bufs=1, space="PSUM")
            pwr = _pwr_cm.__enter__()
            _wpp_cm = tc.tile_pool(name="wrapp", bufs=1)
            wpp = _wpp_cm.__enter__()
            xpad_ap = xpad[:]
            pstride = xpad_ap.ap[0][0]
            base_off = xpad_ap.offset
            offs = xp.tile([18, S], F32)
            _slab_ctr = [0]

            def produce(t0, t1):
                nt = t1 - t0
                # p_conv for rows [2*t0, 2*t1) = c8 groups [t0//4, t1//4)
                for c8 in range(t0 // 4, t1 // 4):
                    acc = pcp.tile([18, 512], F32, tag="pconv")
                    for tap in range(9):
                        dh, dw = tap // 3, tap % 3
                        mov = _ap_raw(xpad_ap,
                                      base_off + (c8 * 8 + dh) * 66 + dw,
                                      [(pstride, 128), (66, 8), (1, 64)])
                        nc.tensor.matmul(acc[:], pwt_b[:, tap, :], mov,
                                         start=(tap == 0), stop=(tap == 8))
                    nc.vector.tensor_copy(offs[:, c8 * 512:(c8 + 1) * 512],
                                          acc[:])
                for t in range(t0, t1):
                    tp = pcp.tile([128, 18], F32, tag="tpose")
                    nc.tensor.transpose(tp[:], offs[:, t * 128:(t + 1) * 128],
                                        id18[:])
                    nc.vector.tensor_copy(posT[:, t, :], tp[:])

                def slab():
                    _slab_ctr[0] += 1
                    return wpp.tile([128, nt, 2 * N], F32,
                                    name=f"slab{_slab_ctr[0]}",
                                    tag=f"slab{_slab_ctr[0]}")

                p4 = slab()
                nc.vector.tensor_add(p4[:], posT[:, t0:t1, :],
                                     base4[:, t0:t1, :])
                pc4 = slab()
                ts(pc4[:], p4[:], 4.0, OP.max, 67.0, OP.min)
                i32 = wpp.tile([128, nt, 2 * N], mybir.dt.int32,
                               name=f"i32_{t0}", tag=f"i32_{t0}")
                nc.vector.tensor_copy(i32[:], p4[:])
                mfr = slab()
                nc.vector.tensor_copy(mfr[:], i32[:])
                f4 = slab()
                nc.vector.tensor_tensor(f4[:], p4[:], mfr[:], op=OP.is_lt)
                nc.vector.tensor_tensor(f4[:], mfr[:], f4[:], op=OP.subtract)
                qlt = mfr                        # reuse
                ts(qlt[:], f4[:], 4.0, OP.max, 67.0, OP.min)
                qrb = slab()
                ts(qrb[:], f4[:], 1.0, OP.add, 4.0, OP.max)
                ts(qrb[:], qrb[:], 67.0, OP.min)
                g04 = slab()
                ts(g04[:], f4[:], 4.0, OP.max, 66.0, OP.min)
                ax = slab()
                nc.vector.tensor_tensor(ax[:], qlt[:], pc4[:], op=OP.subtract)
                ts(ax[:], ax[:], 1.0, OP.add)
                bx = qlt                         # reuse (qlt dead)
                nc.vector.tensor_tensor(bx[:], pc4[:], qrb[:], op=OP.subtract)
                ts(bx[:], bx[:], 1.0, OP.add)
                c1 = pc4                         # reuse (pc4 dead)
                ts(c1[:], f4[:], 66.0, OP.is_le)
                c2 = qrb                         # reuse (qrb dead)
                ts(c2[:], f4[:], 3.0, OP.is_le)
                w0 = f4                          # reuse (f4 dead)
                nc.vector.tensor_tensor(w0[:], ax[:], c1[:], op=OP.mult)
                t0s = slab()
                nc.vector.tensor_tensor(t0s[:], bx[:], c2[:], op=OP.mult)
                nc.vector.tensor_add(w0[:], w0[:], t0s[:])
                w1 = t0s                         # reuse
                nc.vector.tensor_add(w1[:], ax[:], bx[:])
                nc.vector.tensor_tensor(w1[:], w1[:], w0[:], op=OP.subtract)

                # corner weights bf16; (lt, lb, rt, rb)
                wslab = wpp.tile([128, nt, 4, 9], BF16,
                                 name=f"wslab_{t0}", tag=f"wslab_{t0}")
                for j, (wx, wy) in enumerate([(w0, w0), (w1, w0), (w0, w1),
                                              (w1, w1)]):
                    nc.vector.tensor_tensor(wslab[:, :, j, :], wx[:, :, 0:N],
                                            wy[:, :, N:2 * N], op=OP.mult)

                # idx = g04x*64 + g04y - 260 (4-space shift removal)
                idxf = ax                        # reuse (ax dead), x-half
                ts(idxf[:, :, 0:N], g04[:, :, 0:N], 64.0, OP.mult, 260.0,
                   OP.subtract)
                nc.vector.tensor_add(idxf[:, :, 0:N], idxf[:, :, 0:N],
                                     g04[:, :, N:2 * N])
                # route to k-linear DRAM order, k = 9*(128t + p) + n.
                # w: fuse the 4 j-planes into one transpose per n -> tj on
                # partitions; DMA enumerates (t, j, p, n) with inner (p, n)
                # merging to one contiguous 1152-elem run per (t, j).
                wT2 = wpp.tile([nt * 4, 128, 9], BF16,
                               name=f"wT2_{t0}", tag=f"wT2_{t0}")
                iT2 = wpp.tile([nt, 128, 9], F32,
                               name=f"iT2_{t0}", tag=f"iT2_{t0}")
                for n_ in range(9):
                    tpi = pwr.tile([32, 128], F32, tag="tpi")
                    nc.tensor.transpose(tpi[0:nt, :], idxf[:, :, n_],
                                        id128f[:])
                    nc.vector.tensor_copy(iT2[:, :, n_], tpi[0:nt, :])
                    tpw = pwr.tile([128, 128], BF16, tag="tpw")
                    wsl = wslab[:]
                    wmov = _ap_raw(wsl, wsl.offset + n_,
                                   [(wsl.ap[0][0], 128), (36, nt), (9, 4)])
                    nc.tensor.transpose(tpw[0:nt * 4, :], wmov, id128[:])
                    nc.scalar.activation(wT2[:, :, n_], tpw[0:nt * 4, :],
                                         AF.Copy)
                # slice B writes go on the scalar ring so they don't
                # head-of-line-block the first prep's reads on the sync ring
                weng = nc.sync if t0 == 0 else nc.scalar
                weng.dma_start(
                    _ap_raw(idx_dram[:], 1152 * t0,
                            [(1152, nt), (9, 128), (1, 9)]),
                    iT2[:])
                weng.dma_start(
                    _ap_raw(w_dram[:], 1152 * t0,
                            [(1152, nt), (NS, 4), (9, 128), (1, 9)]),
                    wT2[:])

            def emit_prep(gpr, gp, tpp):
                w4pair = gp.tile([128, 128], BF16, tag="w4pair")
                wrd2 = gp.tile([128, 128], BF16, tag="wrd2")
                for g2 in range(2):
                    nc.sync.dma_start(
                        wrd2[g2 * 64:(g2 + 1) * 64, :],
                        _ap_raw(w_dram[:], (2 * gpr + g2) * GH,
                                [(NS, 4), (128, 16), (1, 128)]))
                wps2 = tpp.tile([128, 128], BF16, tag="wps2", bufs=1)
                nc.tensor.transpose(wps2[:], wrd2[:], id128[:])
                nc.scalar.activation(w4pair[:], wps2[:], AF.Copy)
                idxws = []
                for g2 in range(2):
                    g = 2 * gpr + g2
                    idxw = gp.tile([128, GH // 16], I16, tag="idxw",
                                   bufs=4)
                    idr = gp.tile([128, 16], F32, tag="idr", bufs=4)
                    nc.sync.dma_start(
                        idr[:],
                        _ap_raw(idx_dram[:], g * GH, [(16, 128), (1, 16)]))
                    idxT = tpp.tile([16, 128], F32, tag="idxT", bufs=1)
                    nc.tensor.transpose(idxT[:], idr[:], id128f[:])
                    # write stripe 0 via DVE, then log-double across
                    # partitions (3 DMAs instead of 8 replica copies)
                    nc.vector.tensor_copy(idxw[0:16, :], idxT[:])
                    for v in (16, 32, 64):
                        nc.sync.dma_start(idxw[v:2 * v], idxw[0:v])
                    idxws.append(idxw)
                return w4pair, idxws

            produce(0, 8)
            # first two pairs' preps emitted before slice-B production so
            # their PE transposes and sync-ring reads aren't queued behind it
            early_preps = [emit_prep(0, egp, pwr), emit_prep(1, egp, pwr)]
            produce(8, 32)
            _wpp_cm.__exit__(None, None, None)
            _pwr_cm.__exit__(None, None, None)
            _pcp_cm.__exit__(None, None, None)
            _xp_cm.__exit__(None, None, None)

            # gather source AP over x2: overlapping 2-row windows
            x2_src = _ap_raw(x2_d[:], 0, [(2 * INC, S - 1), (1, 4 * INC)])

            # ---------- main loop: gather / combine / PE / y ----------
            y_raw0 = pp.tile([128, NS], BF16)
            chunk_ctr = [0]
            TOT_CHUNKS = NG * NCHUNK
            with tc.tile_pool(name="gather", bufs=2) as gp, \
                 tc.tile_pool(name="xo", bufs=2) as xop, \
                 tc.tile_pool(name="xcmp", bufs=3) as xcp, \
                 tc.tile_pool(name="psum_g", bufs=1, space="PSUM") as gpp, \
                 tc.tile_pool(name="psum_tp", bufs=2, space="PSUM") as tpp, \
                 tc.tile_pool(name="psum_y", bufs=2, space="PSUM") as typ:
                gm1 = gpp.tile([128, 129], F32, tag="gram")

                def emit_pair(gpr, prep, last=False):
                    w4pair, idxws = prep
                    xos = []
                    for g2 in range(2):
                        g = 2 * gpr + g2
                        dst = gp.tile([128, NCHUNK, 512], BF16, tag="gdst", bufs=4)
                        nc.gpsimd.dma_gather(
                            dst[:], x2_src, idxws[g2][:], GH, GH, 4 * INC,
                            elem_step=2 * INC, transpose=False,
                            single_packet=False)

                        # bilinear combine on DVE (stride-0 broadcast weights)
                        dd = dst[:]
                        doff, dstr = dd.offset, dd.ap[0][0]
                        ww = w4pair[:]
                        woff, wstr = ww.offset + g2 * 64, ww.ap[0][0]

                        def ds(j):
                            return _ap_raw(dd, doff + j * 128,
                                           [(dstr, 128), (512, NCHUNK),
                                            (1, 128)])

                        def wb(j):
                            return _ap_raw(ww, woff + j * 16,
                                           [(wstr, 128), (1, 16), (0, 128)])

                        xo = xop.tile([128, NCHUNK, 128], BF16, tag="xo", bufs=3)
                        t0 = xop.tile([128, NCHUNK, 128], BF16, tag="ct0", bufs=2)
                        t1 = xop.tile([128, NCHUNK, 128], BF16, tag="ct1", bufs=2)
                        t2 = xop.tile([128, NCHUNK, 128], BF16, tag="ct2", bufs=2)
                        nc.vector.tensor_tensor(t0[:], ds(0), wb(0), op=OP.mult)
                        nc.vector.tensor_tensor(t1[:], ds(1), wb(1), op=OP.mult)
                        nc.vector.tensor_tensor(t0[:], t0[:], t1[:], op=OP.add)
                        nc.vector.tensor_tensor(t1[:], ds(2), wb(2), op=OP.mult)
                        nc.vector.tensor_tensor(t2[:], ds(3), wb(3), op=OP.mult)
                        nc.vector.tensor_tensor(t1[:], t1[:], t2[:], op=OP.add)
                        nc.vector.tensor_tensor(xo[:], t0[:], t1[:], op=OP.add)
                        xos.append(xo)

                    if last:
                        # close the gram as soon as the combines land: all
                        # gram/m1 matmuls first, transposes/y after
                        for g2 in range(2):
                            xo = xos[g2]
                            for i in range(NCHUNK):
                                ci = chunk_ctr[0]
                                chunk_ctr[0] += 1
                                nc.tensor.matmul(gm1[:, 0:128], xo[:, i, :],
                                                 xo[:, i, :],
                                                 start=(ci == 0),
                                                 stop=(ci == TOT_CHUNKS - 1),
                                                 skip_group_check=True)
                                nc.tensor.matmul(gm1[:, 128:129],
                                                 xo[:, i, :], ones_b[:],
                                                 start=(ci == 0),
                                                 stop=(ci == TOT_CHUNKS - 1),
                                                 skip_group_check=True)
                    for g2 in range(2):
                        g = 2 * gpr + g2
                        xo = xos[g2]
                        # per 4-chunk group: transpose+gram+m1, then y matmuls
                        for grp in range(NCHUNK // 4):
                            xcm = xcp.tile([128, 4, 128], BF16, tag="xcm")
                            for q in range(4):
                                i = grp * 4 + q
                                if not last:
                                    ci = chunk_ctr[0]
                                    chunk_ctr[0] += 1
                                tp = tpp.tile([128, 128], BF16, tag="tp")
                                nc.tensor.transpose(tp[:], xo[:, i, :],
                                                    id128[:])
                                if not last:
                                    nc.tensor.matmul(gm1[:, 0:128],
                                                     xo[:, i, :],
                                                     xo[:, i, :],
                                                     start=(ci == 0),
                                                     stop=(ci == TOT_CHUNKS - 1),
                                                     skip_group_check=True)
                                    nc.tensor.matmul(gm1[:, 128:129],
                                                     xo[:, i, :], ones_b[:],
                                                     start=(ci == 0),
                                                     stop=(ci == TOT_CHUNKS - 1),
                                                     skip_group_check=True)
                                nc.scalar.activation(xcm[:, q, :], tp[:],
                                                     AF.Copy)
                            xmov = xcm[:].rearrange("p a b -> p (a b)")
                            col = g * GH + grp * 512
                            y0p = typ.tile([128, 512], F32, tag="y0",
                                           bufs=1)
                            nc.tensor.matmul(y0p[:], cwt_b[:, 0:128], xmov,
                                             start=True, stop=True)
                            y1p = typ.tile([128, 512], F32, tag="y1", bufs=1)
                            nc.tensor.matmul(y1p[:], cwt_b[:, 128:256], xmov,
                                             start=True, stop=True)
                            nc.scalar.activation(y_raw0[:, col:col + 512],
                                                 y0p[:], AF.Copy)
                            ystg = xcp.tile([128, 512], BF16, tag="ystg")
                            nc.scalar.activation(ystg[:], y1p[:], AF.Copy)
                            nc.sync.dma_start(y1_dram[:, col:col + 512],
                                              ystg[:])

                def local_stats(g_sb, tag):
                    # stat_l = [cwt^T G cwt diag | m1^T cwt], linear in G/m1
                    t1p = typ.tile([128, 512], F32, tag="y0", bufs=1)
                    nc.tensor.matmul(t1p[:, 0:OUTC], g_sb[:, 0:128], cwt[:],
                                     start=True, stop=True)
                    m2 = wp.tile([128, OUTC], F32, name=f"m2{tag}")
                    nc.vector.tensor_tensor(m2[:], cwt[:], t1p[:, 0:OUTC],
                                            op=OP.mult)
                    dvm1 = typ.tile([128, 512], F32, tag="y0", bufs=1)
                    nc.tensor.matmul(dvm1[0:1, 0:OUTC], ones_f[:], m2[:],
                                     start=True, stop=True)
                    nc.tensor.matmul(dvm1[0:1, OUTC:2 * OUTC],
                                     g_sb[:, 128:129], cwt[:],
                                     start=True, stop=True)
                    stat_l = wp.tile([1, 2 * OUTC], F32, name=f"sl{tag}")
                    nc.vector.tensor_copy(stat_l[:], dvm1[0:1, :])
                    stat = wp.tile([1, 2 * OUTC], F32, name=f"st{tag}")
                    if stage >= 3:
                        b_in = dp.tile([1, 2 * OUTC], F32)
                        b_out = dp.tile([1, 2 * OUTC], F32,
                                        addr_space="Shared")
                        nc.sync.dma_start(b_in[:], stat_l[:])
                        nc.gpsimd.collective_compute(
                            "AllReduce", OP.add,
                            replica_groups=[list(range(N_CORES))],
                            ins=[b_in[:].opt()],
                            outs=[b_out[:].opt()])
                        nc.sync.dma_start(stat[:], b_out[:])
                    else:
                        nc.vector.tensor_scalar(stat[:], stat_l[:], 8.0,
                                                None, op0=OP.mult)
                    return stat

                preps = {0: early_preps[0], 1: early_preps[1]}
                for gpr in range(NG // 2):
                    if gpr + 1 < NG // 2 and gpr + 1 not in preps:
                        preps[gpr + 1] = emit_prep(gpr + 1, gp, tpp)
                    emit_pair(gpr, preps.pop(gpr), last=(gpr == NG // 2 - 1))
                    if chunk_ctr[0] == SPLIT_CHUNK and stage >= 3:
                        # dummy allreduce: aligns cores mid-loop (CC runs on
                        # the CC cores; gpsimd only triggers) so the real
                        # stats allreduce at the end has no skew wait
                        d_in = dp.tile([1, 2 * OUTC], F32)
                        d_out = dp.tile([1, 2 * OUTC], F32,
                                        addr_space="Shared")
                        nc.scalar.dma_start(d_in[:], gb_d[:])
                        nc.gpsimd.collective_compute(
                            "AllReduce", OP.add,
                            replica_groups=[list(range(N_CORES))],
                            ins=[d_in[:].opt()],
                            outs=[d_out[:].opt()])
                g_sb = wp.tile([128, 129], F32)
                nc.vector.tensor_copy(g_sb[:], gm1[:])
                stat = local_stats(g_sb, "F")
            _egp_cm.__exit__(None, None, None)

            # ---------- BN coefficients ----------
            with tc.tile_pool(name="psum_s", bufs=1, space="PSUM") as sp:
                meanv = wp.tile([1, OUTC], F32)
                ts(meanv[:], stat[:, OUTC:2 * OUTC], 1.0 / S_TOT, OP.mult)
                varv = wp.tile([1, OUTC], F32)
                ts(varv[:], stat[:, 0:OUTC], 1.0 / S_TOT, OP.mult)
                msq = wp.tile([1, OUTC], F32)
                nc.vector.tensor_tensor(msq[:], meanv[:], meanv[:], op=OP.mult)
                nc.vector.tensor_tensor(varv[:], varv[:], msq[:],
                                        op=OP.subtract)
                ts(varv[:], varv[:], EPS, OP.add)
                sd = wp.tile([1, OUTC], F32)
                nc.scalar.activation(sd[:], varv[:], AF.Sqrt)
                rsd = wp.tile([1, OUTC], F32)
                nc.vector.reciprocal(rsd[:], sd[:])
                a_v = wp.tile([1, OUTC], F32)
                nc.vector.tensor_tensor(a_v[:], rsd[:], gb[:, 0:OUTC],
                                        op=OP.mult)
                b_v = wp.tile([1, OUTC], F32)
                nc.vector.tensor_tensor(b_v[:], meanv[:], a_v[:], op=OP.mult)
                nc.vector.tensor_tensor(b_v[:], gb[:, OUTC:2 * OUTC], b_v[:],
                                        op=OP.subtract)

                # transpose a/b rows to per-partition layout on PE
                ab_ps = sp.tile([128, 4], F32)
                for ci_, (vec, lo) in enumerate([(a_v, 0), (b_v, 0),
                                                 (a_v, 128), (b_v, 128)]):
                    col = (ci_ % 2) + (ci_ // 2) * 2
                    nc.tensor.matmul(ab_ps[:, col:col + 1],
                                     vec[:, lo:lo + 128], ones_f[0:1, 0:1],
                                     start=True, stop=True)
                ab = pp.tile([128, 2, 2], F32)
                nc.vector.tensor_copy(ab[:].rearrange("p a b -> p (a b)"),
                                      ab_ps[:])

            # ---------- silu epilogue ----------
            YB = 4096
            with tc.tile_pool(name="ybuf", bufs=4) as yb, \
                 tc.tile_pool(name="y1rd", bufs=7) as yr:
                y1t = []
                for blk in range(NS // YB):
                    y1blk = yr.tile([128, YB], BF16, tag="y1blk")
                    nc.scalar.dma_start(y1blk[:],
                                        y1_dram[:, blk * YB:(blk + 1) * YB])
                    y1t.append(y1blk)
                    ybuf = yb.tile([128, YB], BF16, tag="yb")
                    nc.scalar.activation(
                        ybuf[:], y_raw0[:, blk * YB:(blk + 1) * YB],
                        AF.Silu, scale=ab[:, 0, 0:1], bias=ab[:, 0, 1:2])
                    out_ap = _ap_raw(
                        out_d[:], blk * YB,
                        [(NS, 128), (1, YB)])
                    nc.sync.dma_start(out_ap, ybuf[:])
                    ybuf1 = yb.tile([128, YB], BF16, tag="yb")
                    nc.scalar.activation(
                        ybuf1[:], y1blk[:],
                        AF.Silu, scale=ab[:, 1, 0:1], bias=ab[:, 1, 1:2])
                    out_ap1 = _ap_raw(
                        out_d[:], 128 * NS + blk * YB,
                        [(NS, 128), (1, YB)])
                    nc.sync.dma_start(out_ap1, ybuf1[:])

    nc.compile()
    return nc


def prep_inputs(x, pw, pb, cw, gamma, beta):
    x = np.asarray(x, np.float32)
    pw = np.asarray(pw, np.float32)
    pb = np.asarray(pb, np.float32)
    cw = np.asarray(cw, np.float32)
    gamma = np.asarray(gamma, np.float32)
    beta = np.asarray(beta, np.float32)

    pwt = np.ascontiguousarray(
        pw.reshape(2 * N, INC, 9).transpose(1, 2, 0))      # (128, 9, 18)

    angles = np.linspace(0.0, 2.0 * math.pi, N + 1, dtype=np.float64)[:-1]
    pn = np.concatenate([np.cos(angles), np.sin(angles)]).astype(np.float32)
    p_idx = np.arange(128)
    t_idx = np.arange(32)
    hh = (2 * t_idx[None, :] + (p_idx[:, None] >= 64)).astype(np.float32)
    ww = np.broadcast_to((p_idx % 64).astype(np.float32)[:, None], (128, 32))
    base4 = np.zeros((128, 32, 2 * N), np.float32)
    base4[:, :, 0:N] = hh[:, :, None] + (pn[0:N] + pb[0:N])[None, None, :] + 4.0
    base4[:, :, N:] = ww[:, :, None] + (pn[N:] + pb[N:])[None, None, :] + 4.0

    cwt = np.ascontiguousarray(cw[:, :, 0, 0].T)           # (128, 256)
    gb = np.concatenate([gamma, beta])[None, :]            # (1, 512)
    id128 = np.eye(128, dtype=np.float32).astype(ml_dtypes.bfloat16)

    in_maps = []
    for b in range(B):
        xb = x[b].reshape(INC, S)
        xpad = np.zeros((INC, 66, 66), ml_dtypes.bfloat16)
        xpad[:, 1:65, 1:65] = x[b]
        xT = np.ascontiguousarray(xb.T).astype(ml_dtypes.bfloat16)  # (4096, 128)
        x2 = np.zeros((S, 2 * INC), ml_dtypes.bfloat16)
        x2[:, 0:INC] = xT
        x2[:S - 64, INC:] = xT[64:]
        in_maps.append(dict(
            xpad=np.ascontiguousarray(xpad.reshape(INC, 66 * 66)), x2=x2,
            pwt=pwt, base4=base4, cwt=cwt, gb=gb,
            id18=np.eye(18, dtype=np.float32), id128=id128,
            id128f=np.eye(128, dtype=np.float32)))
    return in_maps


_NC_CACHE = {}


def kernel(x, pw, pb, cw, gamma, beta):
    import os
    if "nc" not in _NC_CACHE:
        _NC_CACHE["nc"] = build(
            new_idx=os.environ.get("NEWIDX", "1") == "1",
            new_w=os.environ.get("NEWW", "1") == "1")
    nc = _NC_CACHE["nc"]
    in_maps = prep_inputs(x, pw, pb, cw, gamma, beta)
    res = run_bass_kernel_spmd(nc, in_maps, core_ids=list(range(N_CORES)))
    out = np.stack([
        np.asarray(res.results[b]["out"]).astype(np.float32).reshape(
            OUTC, H, W * N)
        for b in range(B)])
    return out



# revision 59
# speedup vs baseline: 1.0043x; 1.0043x over previous
"""AKConv TRN2 kernel: 8-core data-parallel over batch.

Sample-major gather architecture: dma_gather(transpose=False) puts each
sample (4 corners x 128ch) on one partition; bilinear combine on DVE via
stride-0 broadcast weights; PE transposes chunks back to channel-major,
accumulates the BN gram + row sums off the same stationary, and runs the
1x1 conv for both outc halves inline. Tail: allreduce -> BN coeffs ->
SiLU -> output DMA.
"""
import sys
sys.path.insert(0, "/opt/trn_rl_repo")
import math
import numpy as np
import ml_dtypes
import bass_rust
import concourse.bass as bass
import concourse.tile as tile
from concourse import bacc, mybir
from concourse.bass_utils import run_bass_kernel_spmd

F32 = mybir.dt.float32
BF16 = mybir.dt.bfloat16
I16 = mybir.dt.int16
AF = mybir.ActivationFunctionType
OP = mybir.AluOpType

B, INC, H, W = 8, 128, 64, 64
OUTC, N = 256, 9
S = H * W                      # 4096 pixels per core
NS = N * S                     # 36864 samples per core
S_TOT = float(B * NS)          # BN sample count
EPS = 1e-5
N_CORES = 8

GH = 2048                      # samples per dma_gather
NG = NS // GH                  # 18 gathers
NCHUNK = GH // 128             # 16 chunks of 128 samples per gather
SPLIT_CHUNK = 256              # close gram A after pair 7 (8/9 of chunks)


def _ap_raw(ap, offset, dims):
    a = ap.copy()
    a.offset = offset
    a.ap = bass_rust.VecI64Pair(dims)
    return a


def build(stage=3, new_idx=True, new_w=True):
    nc = bacc.Bacc("TRN2", target_bir_lowering=False, debug=False,
                   num_devices=N_CORES)
    xpad_d = nc.dram_tensor("xpad", [INC, 66 * 66], BF16, kind="ExternalInput")
    x2_d = nc.dram_tensor("x2", [S, 2 * INC], BF16, kind="ExternalInput")
    pwt_d = nc.dram_tensor("pwt", [INC, 9, 2 * N], F32, kind="ExternalInput")
    base4_d = nc.dram_tensor("base4", [128, 32, 2 * N], F32, kind="ExternalInput")
    cwt_d = nc.dram_tensor("cwt", [INC, OUTC], F32, kind="ExternalInput")
    gb_d = nc.dram_tensor("gb", [1, 2 * OUTC], F32, kind="ExternalInput")
    id18_d = nc.dram_tensor("id18", [18, 18], F32, kind="ExternalInput")
    id128_d = nc.dram_tensor("id128", [128, 128], BF16, kind="ExternalInput")
    id128f_d = nc.dram_tensor("id128f", [128, 128], F32, kind="ExternalInput")
    out_d = nc.dram_tensor("out", [OUTC, NS], BF16, kind="ExternalOutput")

    idx_dram = nc.dram_tensor("idx_scratch", [1, NS], F32, kind="Internal")
    w_dram = nc.dram_tensor("w_scratch", [4, NS], BF16, kind="Internal")
    ab_dram = nc.dram_tensor("ab_scratch", [OUTC, 2], F32, kind="Internal")
    y1_dram = nc.dram_tensor("y1_scratch", [128, NS], BF16, kind="Internal")

    with tile.TileContext(nc) as tc:
        with tc.tile_pool(name="persist", bufs=1) as pp, \
             tc.tile_pool(name="work", bufs=1) as wp, \
             tc.tile_pool(name="dram", bufs=1, space="DRAM") as dp:

            # ---------- loads (spread across HWDGE rings, xpad first) ----------
            warm_in = dp.tile([1, 4], F32)
            warm_out = dp.tile([1, 4], F32, addr_space="Shared")
            if stage >= 3:
                nc.sync.dma_start(warm_in[:], gb_d[:, 0:4])
                nc.gpsimd.collective_compute(
                    "AllReduce", OP.add,
                    replica_groups=[list(range(N_CORES))],
                    ins=[warm_in[:].opt()],
                    outs=[warm_out[:].opt()])
            _egp_cm = tc.tile_pool(name="earlygp", bufs=2)
            egp = _egp_cm.__enter__()
            _xp_cm = tc.tile_pool(name="xpadp", bufs=1)
            xp = _xp_cm.__enter__()
            xpad = xp.tile([INC, 66 * 66], BF16)
            nc.sync.dma_start(xpad[:], xpad_d[:])
            pwt = pp.tile([INC, 9, 2 * N], F32)
            nc.scalar.dma_start(pwt[:], pwt_d[:])
            pwt_b = pp.tile([INC, 9, 2 * N], BF16)
            nc.vector.tensor_copy(pwt_b[:], pwt[:])
            id18 = pp.tile([18, 18], F32)
            nc.scalar.dma_start(id18[:], id18_d[:])
            id128 = pp.tile([128, 128], BF16)
            nc.scalar.dma_start(id128[:], id128_d[:])
            id128f = pp.tile([128, 128], F32)
            nc.scalar.dma_start(id128f[:], id128f_d[:])
            base4 = pp.tile([128, 32, 2 * N], F32)
            nc.scalar.dma_start(base4[:], base4_d[:])
            cwt = pp.tile([INC, OUTC], F32)
            nc.scalar.dma_start(cwt[:], cwt_d[:])
            gb = pp.tile([1, 2 * OUTC], F32)
            nc.scalar.dma_start(gb[:], gb_d[:])
            cwt_b = pp.tile([INC, OUTC], BF16)
            nc.vector.tensor_copy(cwt_b[:], cwt[:])
            ones_b = pp.tile([128, 1], BF16)
            nc.vector.memset(ones_b[:], 1.0)
            ones_f = pp.tile([128, 1], F32)
            nc.vector.memset(ones_f[:], 1.0)
            posT = pp.tile([128, 32, 2 * N], F32)

            # ---------- sliced prologue: produce t-slabs [t0, t1) ----------
            def ts(out, in_, s1, o1, s2=None, o2=None):
                if s2 is None:
                    nc.vector.tensor_scalar(out, in_, s1, None, op0=o1)
                else:
                    nc.vector.tensor_scalar(out, in_, s1, s2, op0=o1, op1=o2)

            _pcp_cm = tc.tile_pool(name="psum_pc", bufs=2, space="PSUM")
            pcp = _pcp_cm.__enter__()
            _pwr_cm = tc.tile_pool(name="psum_wr", bufs=1, space="PSUM")
            pwr = _pwr_cm.__enter__()
            _wpp_cm = tc.tile_pool(name="wrapp", bufs=1)
            wpp = _wpp_cm.__enter__()
            xpad_ap = xpad[:]
            pstride = xpad_ap.ap[0][0]
            base_off = xpad_ap.offset
            offs = xp.tile([18, S], F32)
            _slab_ctr = [0]

            def produce(t0, t1):
                nt = t1 - t0
                # p_conv for rows [2*t0, 2*t1) = c8 groups [t0//4, t1//4)
                for c8 in range(t0 // 4, t1 // 4):
                    acc = pcp.tile([18, 512], F32, tag="pconv")
                    for tap in range(9):
                        dh, dw = tap // 3, tap % 3
                        mov = _ap_raw(xpad_ap,
                                      base_off + (c8 * 8 + dh) * 66 + dw,
                                      [(pstride, 128), (66, 8), (1, 64)])
                        nc.tensor.matmul(acc[:], pwt_b[:, tap, :], mov,
                                         start=(tap == 0), stop=(tap == 8))
                    nc.vector.tensor_copy(offs[:, c8 * 512:(c8 + 1) * 512],
                                          acc[:])
                for t in range(t0, t1):
                    tp = pcp.tile([128, 18], F32, tag="tpose")
                    nc.tensor.transpose(tp[:], offs[:, t * 128:(t + 1) * 128],
                                        id18[:])
                    nc.vector.tensor_copy(posT[:, t, :], tp[:])

                def slab():
                    _slab_ctr[0] += 1
                    return wpp.tile([128, nt, 2 * N], F32,
                                    name=f"slab{_slab_ctr[0]}",
                                    tag=f"slab{_slab_ctr[0]}")

                p4 = slab()
                nc.vector.tensor_add(p4[:], posT[:, t0:t1, :],
                                     base4[:, t0:t1, :])
                pc4 = slab()
                ts(pc4[:], p4[:], 4.0, OP.max, 67.0, OP.min)
                i32 = wpp.tile([128, nt, 2 * N], mybir.dt.int32,
                               name=f"i32_{t0}", tag=f"i32_{t0}")
                nc.vector.tensor_copy(i32[:], p4[:])
                mfr = slab()
                nc.vector.tensor_copy(mfr[:], i32[:])
                f4 = slab()
                nc.vector.tensor_tensor(f4[:], p4[:], mfr[:], op=OP.is_lt)
                nc.vector.tensor_tensor(f4[:], mfr[:], f4[:], op=OP.subtract)
                qlt = mfr                        # reuse
                ts(qlt[:], f4[:], 4.0, OP.max, 67.0, OP.min)
                qrb = slab()
                ts(qrb[:], f4[:], 1.0, OP.add, 4.0, OP.max)
                ts(qrb[:], qrb[:], 67.0, OP.min)
                g04 = slab()
                ts(g04[:], f4[:], 4.0, OP.max, 66.0, OP.min)
                ax = slab()
                nc.vector.tensor_tensor(ax[:], qlt[:], pc4[:], op=OP.subtract)
                ts(ax[:], ax[:], 1.0, OP.add)
                bx = qlt                         # reuse (qlt dead)
                nc.vector.tensor_tensor(bx[:], pc4[:], qrb[:], op=OP.subtract)
                ts(bx[:], bx[:], 1.0, OP.add)
                c1 = pc4                         # reuse (pc4 dead)
                ts(c1[:], f4[:], 66.0, OP.is_le)
                c2 = qrb                         # reuse (qrb dead)
                ts(c2[:], f4[:], 3.0, OP.is_le)
                w0 = f4                          # reuse (f4 dead)
                nc.vector.tensor_tensor(w0[:], ax[:], c1[:], op=OP.mult)
                t0s = slab()
                nc.vector.tensor_tensor(t0s[:], bx[:], c2[:], op=OP.mult)
                nc.vector.tensor_add(w0[:], w0[:], t0s[:])
                w1 = t0s                         # reuse
                nc.vector.tensor_add(w1[:], ax[:], bx[:])
                nc.vector.tensor_tensor(w1[:], w1[:], w0[:], op=OP.subtract)

                # corner weights bf16; (lt, lb, rt, rb)
                wslab = wpp.tile([128, nt, 4, 9], BF16,
                                 name=f"wslab_{t0}", tag=f"wslab_{t0}")
                for j, (wx, wy) in enumerate([(w0, w0), (w1, w0), (w0, w1),
                                              (w1, w1)]):
                    nc.vector.tensor_tensor(wslab[:, :, j, :], wx[:, :, 0:N],
                                            wy[:, :, N:2 * N], op=OP.mult)

                # idx = g04x*64 + g04y - 260 (4-space shift removal)
                idxf = ax                        # reuse (ax dead), x-half
                ts(idxf[:, :, 0:N], g04[:, :, 0:N], 64.0, OP.mult, 260.0,
                   OP.subtract)
                nc.vector.tensor_add(idxf[:, :, 0:N], idxf[:, :, 0:N],
                                     g04[:, :, N:2 * N])
                # route to k-linear DRAM order, k = 9*(128t + p) + n.
                # w: fuse the 4 j-planes into one transpose per n -> tj on
                # partitions; DMA enumerates (t, j, p, n) with inner (p, n)
                # merging to one contiguous 1152-elem run per (t, j).
                wT2 = wpp.tile([nt * 4, 128, 9], BF16,
                               name=f"wT2_{t0}", tag=f"wT2_{t0}")
                iT2 = wpp.tile([nt, 128, 9], F32,
                               name=f"iT2_{t0}", tag=f"iT2_{t0}")
                for n_ in range(9):
                    tpi = pwr.tile([32, 128], F32, tag="tpi")
                    nc.tensor.transpose(tpi[0:nt, :], idxf[:, :, n_],
                                        id128f[:])
                    nc.vector.tensor_copy(iT2[:, :, n_], tpi[0:nt, :])
                    tpw = pwr.tile([128, 128], BF16, tag="tpw")
                    wsl = wslab[:]
                    wmov = _ap_raw(wsl, wsl.offset + n_,
                                   [(wsl.ap[0][0], 128), (36, nt), (9, 4)])
                    nc.tensor.transpose(tpw[0:nt * 4, :], wmov, id128[:])
                    nc.scalar.activation(wT2[:, :, n_], tpw[0:nt * 4, :],
                                         AF.Copy)
                # slice B writes go on the scalar ring so they don't
                # head-of-line-block the first prep's reads on the sync ring
                weng = nc.sync if t0 == 0 else nc.scalar
                weng.dma_start(
                    _ap_raw(idx_dram[:], 1152 * t0,
                            [(1152, nt), (9, 128), (1, 9)]),
                    iT2[:])
                weng.dma_start(
                    _ap_raw(w_dram[:], 1152 * t0,
                            [(1152, nt), (NS, 4), (9, 128), (1, 9)]),
                    wT2[:])

            def emit_prep(gpr, gp, tpp):
                w4pair = gp.tile([128, 128], BF16, tag="w4pair")
                wrd2 = gp.tile([128, 128], BF16, tag="wrd2")
                for g2 in range(2):
                    nc.sync.dma_start(
                        wrd2[g2 * 64:(g2 + 1) * 64, :],
                        _ap_raw(w_dram[:], (2 * gpr + g2) * GH,
                                [(NS, 4), (128, 16), (1, 128)]))
                wps2 = tpp.tile([128, 128], BF16, tag="wps2", bufs=1)
                nc.tensor.transpose(wps2[:], wrd2[:], id128[:])
                nc.scalar.activation(w4pair[:], wps2[:], AF.Copy)
                idxws = []
                for g2 in range(2):
                    g = 2 * gpr + g2
                    idxw = gp.tile([128, GH // 16], I16, tag="idxw",
                                   bufs=4)
                    idr = gp.tile([128, 16], F32, tag="idr", bufs=4)
                    nc.sync.dma_start(
                        idr[:],
                        _ap_raw(idx_dram[:], g * GH, [(16, 128), (1, 16)]))
                    idxT = tpp.tile([16, 128], F32, tag="idxT", bufs=1)
                    nc.tensor.transpose(idxT[:], idr[:], id128f[:])
                    # write stripe 0 via DVE, then log-double across
                    # partitions (3 DMAs instead of 8 replica copies)
                    nc.vector.tensor_copy(idxw[0:16, :], idxT[:])
                    for v in (16, 32, 64):
                        nc.sync.dma_start(idxw[v:2 * v], idxw[0:v])
                    idxws.append(idxw)
                return w4pair, idxws

            produce(0, 8)
            # first two pairs' preps emitted before slice-B production so
            # their PE transposes and sync-ring reads aren't queued behind it
            early_preps = [emit_prep(0, egp, pwr), emit_prep(1, egp, pwr)]
            produce(8, 32)
            _wpp_cm.__exit__(None, None, None)
            _pwr_cm.__exit__(None, None, None)
            _pcp_cm.__exit__(None, None, None)
            _xp_cm.__exit__(None, None, None)

            # gather source AP over x2: overlapping 2-row windows
            x2_src = _ap_raw(x2_d[:], 0, [(2 * INC, S - 1), (1, 4 * INC)])

            # ---------- main loop: gather / combine / PE / y ----------
            y_raw0 = pp.tile([128, NS], BF16)
            chunk_ctr = [0]
            TOT_CHUNKS = NG * NCHUNK
            with tc.tile_pool(name="gather", bufs=2) as gp, \
                 tc.tile_pool(name="xo", bufs=2) as xop, \
                 tc.tile_pool(name="xcmp", bufs=3) as xcp, \
                 tc.tile_pool(name="psum_g", bufs=1, space="PSUM") as gpp, \
                 tc.tile_pool(name="psum_tp", bufs=2, space="PSUM") as tpp, \
                 tc.tile_pool(name="psum_y", bufs=2, space="PSUM") as typ:
                gm1 = gpp.tile([128, 129], F32, tag="gram")

                def emit_pair(gpr, prep, last=False):
                    w4pair, idxws = prep
                    xos = []
                    for g2 in range(2):
                        g = 2 * gpr + g2
                        dst = gp.tile([128, NCHUNK, 512], BF16, tag="gdst", bufs=4)
                        nc.gpsimd.dma_gather(
                            dst[:], x2_src, idxws[g2][:], GH, GH, 4 * INC,
                            elem_step=2 * INC, transpose=False,
                            single_packet=False)

                        # bilinear combine on DVE (stride-0 broadcast weights)
                        dd = dst[:]
                        doff, dstr = dd.offset, dd.ap[0][0]
                        ww = w4pair[:]
                        woff, wstr = ww.offset + g2 * 64, ww.ap[0][0]

                        def ds(j):
                            return _ap_raw(dd, doff + j * 128,
                                           [(dstr, 128), (512, NCHUNK),
                                            (1, 128)])

                        def wb(j):
                            return _ap_raw(ww, woff + j * 16,
                                           [(wstr, 128), (1, 16), (0, 128)])

                        xo = xop.tile([128, NCHUNK, 128], BF16, tag="xo", bufs=3)
                        t0 = xop.tile([128, NCHUNK, 128], BF16, tag="ct0", bufs=2)
                        t1 = xop.tile([128, NCHUNK, 128], BF16, tag="ct1", bufs=2)
                        t2 = xop.tile([128, NCHUNK, 128], BF16, tag="ct2", bufs=2)
                        nc.vector.tensor_tensor(t0[:], ds(0), wb(0), op=OP.mult)
                        nc.vector.tensor_tensor(t1[:], ds(1), wb(1), op=OP.mult)
                        nc.vector.tensor_tensor(t0[:], t0[:], t1[:], op=OP.add)
                        nc.vector.tensor_tensor(t1[:], ds(2), wb(2), op=OP.mult)
                        nc.vector.tensor_tensor(t2[:], ds(3), wb(3), op=OP.mult)
                        nc.vector.tensor_tensor(t1[:], t1[:], t2[:], op=OP.add)
                        nc.vector.tensor_tensor(xo[:], t0[:], t1[:], op=OP.add)
                        xos.append(xo)

                    if last:
                        # close the gram as soon as the combines land: all
                        # gram/m1 matmuls first, transposes/y after
                        for g2 in range(2):
                            xo = xos[g2]
                            for i in range(NCHUNK):
                                ci = chunk_ctr[0]
                                chunk_ctr[0] += 1
                                nc.tensor.matmul(gm1[:, 0:128], xo[:, i, :],
                                                 xo[:, i, :],
                                                 start=(ci == 0),
                                                 stop=(ci == TOT_CHUNKS - 1),
                                                 skip_group_check=True)
                                nc.tensor.matmul(gm1[:, 128:129],
                                                 xo[:, i, :], ones_b[:],
                                                 start=(ci == 0),
                                                 stop=(ci == TOT_CHUNKS - 1),
                                                 skip_group_check=True)
                    for g2 in range(2):
                        g = 2 * gpr + g2
                        xo = xos[g2]
                        # per 4-chunk group: transpose+gram+m1, then y matmuls
                        for grp in range(NCHUNK // 4):
                            xcm = xcp.tile([128, 4, 128], BF16, tag="xcm")
                            for q in range(4):
                                i = grp * 4 + q
                                if not last:
                                    ci = chunk_ctr[0]
                                    chunk_ctr[0] += 1
                                tp = tpp.tile([128, 128], BF16, tag="tp")
                                nc.tensor.transpose(tp[:], xo[:, i, :],
                                                    id128[:])
                                if not last:
                                    nc.tensor.matmul(gm1[:, 0:128],
                                                     xo[:, i, :],
                                                     xo[:, i, :],
                                                     start=(ci == 0),
                                                     stop=(ci == TOT_CHUNKS - 1),
                                                     skip_group_check=True)
                                    nc.tensor.matmul(gm1[:, 128:129],
                                                     xo[:, i, :], ones_b[:],
                                                     start=(ci == 0),
                                                     stop=(ci == TOT_CHUNKS - 1),
                                                     skip_group_check=True)
                                nc.scalar.activation(xcm[:, q, :], tp[:],
                                                     AF.Copy)
                            xmov = xcm[:].rearrange("p a b -> p (a b)")
                            col = g * GH + grp * 512
                            y0p = typ.tile([128, 512], F32, tag="y0",
                                           bufs=1)
                            nc.tensor.matmul(y0p[:], cwt_b[:, 0:128], xmov,
                                             start=True, stop=True)
                            y1p = typ.tile([128, 512], F32, tag="y1", bufs=1)
                            nc.tensor.matmul(y1p[:], cwt_b[:, 128:256], xmov,
                                             start=True, stop=True)
                            nc.scalar.activation(y_raw0[:, col:col + 512],
                                                 y0p[:], AF.Copy)
                            ystg = xcp.tile([128, 512], BF16, tag="ystg")
                            nc.scalar.activation(ystg[:], y1p[:], AF.Copy)
                            nc.sync.dma_start(y1_dram[:, col:col + 512],
                                              ystg[:])

                def local_stats(g_sb, tag):
                    # stat_l = [cwt^T G cwt diag | m1^T cwt], linear in G/m1
                    t1p = typ.tile([128, 512], F32, tag="y0", bufs=1)
                    nc.tensor.matmul(t1p[:, 0:OUTC], g_sb[:, 0:128], cwt[:],
                                     start=True, stop=True)
                    m2 = wp.tile([128, OUTC], F32, name=f"m2{tag}")
                    nc.vector.tensor_tensor(m2[:], cwt[:], t1p[:, 0:OUTC],
                                            op=OP.mult)
                    dvm1 = typ.tile([128, 512], F32, tag="y0", bufs=1)
                    nc.tensor.matmul(dvm1[0:1, 0:OUTC], ones_f[:], m2[:],
                                     start=True, stop=True)
                    nc.tensor.matmul(dvm1[0:1, OUTC:2 * OUTC],
                                     g_sb[:, 128:129], cwt[:],
                                     start=True, stop=True)
                    stat_l = wp.tile([1, 2 * OUTC], F32, name=f"sl{tag}")
                    nc.vector.tensor_copy(stat_l[:], dvm1[0:1, :])
                    stat = wp.tile([1, 2 * OUTC], F32, name=f"st{tag}")
                    if stage >= 3:
                        b_in = dp.tile([1, 2 * OUTC], F32)
                        b_out = dp.tile([1, 2 * OUTC], F32,
                                        addr_space="Shared")
                        nc.sync.dma_start(b_in[:], stat_l[:])
                        nc.gpsimd.collective_compute(
                            "AllReduce", OP.add,
                            replica_groups=[list(range(N_CORES))],
                            ins=[b_in[:].opt()],
                            outs=[b_out[:].opt()])
                        nc.sync.dma_start(stat[:], b_out[:])
                    else:
                        nc.vector.tensor_scalar(stat[:], stat_l[:], 8.0,
                                                None, op0=OP.mult)
                    return stat

                preps = {0: early_preps[0], 1: early_preps[1]}
                for gpr in range(NG // 2):
                    if gpr + 1 < NG // 2 and gpr + 1 not in preps:
                        preps[gpr + 1] = emit_prep(gpr + 1, gp, tpp)
                    emit_pair(gpr, preps.pop(gpr))
                    if chunk_ctr[0] == SPLIT_CHUNK and stage >= 3:
                        # dummy allreduce: aligns cores mid-loop (CC runs on
                        # the CC cores; gpsimd only triggers) so the real
                        # stats allreduce at the end has no skew wait
                        d_in = dp.tile([1, 2 * OUTC], F32)
                        d_out = dp.tile([1, 2 * OUTC], F32,
                                        addr_space="Shared")
                        nc.scalar.dma_start(d_in[:], gb_d[:])
                        nc.gpsimd.collective_compute(
                            "AllReduce", OP.add,
                            replica_groups=[list(range(N_CORES))],
                            ins=[d_in[:].opt()],
                            outs=[d_out[:].opt()])
                g_sb = wp.tile([128, 129], F32)
                nc.vector.tensor_copy(g_sb[:], gm1[:])
                stat = local_stats(g_sb, "F")
            _egp_cm.__exit__(None, None, None)

            # ---------- BN coefficients ----------
            with tc.tile_pool(name="psum_s", bufs=1, space="PSUM") as sp:
                meanv = wp.tile([1, OUTC], F32)
                ts(meanv[:], stat[:, OUTC:2 * OUTC], 1.0 / S_TOT, OP.mult)
                varv = wp.tile([1, OUTC], F32)
                ts(varv[:], stat[:, 0:OUTC], 1.0 / S_TOT, OP.mult)
                msq = wp.tile([1, OUTC], F32)
                nc.vector.tensor_tensor(msq[:], meanv[:], meanv[:], op=OP.mult)
                nc.vector.tensor_tensor(varv[:], varv[:], msq[:],
                                        op=OP.subtract)
                ts(varv[:], varv[:], EPS, OP.add)
                sd = wp.tile([1, OUTC], F32)
                nc.scalar.activation(sd[:], varv[:], AF.Sqrt)
                rsd = wp.tile([1, OUTC], F32)
                nc.vector.reciprocal(rsd[:], sd[:])
                a_v = wp.tile([1, OUTC], F32)
                nc.vector.tensor_tensor(a_v[:], rsd[:], gb[:, 0:OUTC],
                                        op=OP.mult)
                b_v = wp.tile([1, OUTC], F32)
                nc.vector.tensor_tensor(b_v[:], meanv[:], a_v[:], op=OP.mult)
                nc.vector.tensor_tensor(b_v[:], gb[:, OUTC:2 * OUTC], b_v[:],
                                        op=OP.subtract)

                # transpose a/b rows to per-partition layout on PE
                ab_ps = sp.tile([128, 4], F32)
                for ci_, (vec, lo) in enumerate([(a_v, 0), (b_v, 0),
                                                 (a_v, 128), (b_v, 128)]):
                    col = (ci_ % 2) + (ci_ // 2) * 2
                    nc.tensor.matmul(ab_ps[:, col:col + 1],
                                     vec[:, lo:lo + 128], ones_f[0:1, 0:1],
                                     start=True, stop=True)
                ab = pp.tile([128, 2, 2], F32)
                nc.vector.tensor_copy(ab[:].rearrange("p a b -> p (a b)"),
                                      ab_ps[:])

            # ---------- silu epilogue ----------
            YB = 4096
            with tc.tile_pool(name="ybuf", bufs=4) as yb, \
                 tc.tile_pool(name="y1rd", bufs=7) as yr:
                y1t = []
                for blk in range(NS // YB):
                    y1blk = yr.tile([128, YB], BF16, tag="y1blk")
                    nc.scalar.dma_start(y1blk[:],
                                        y1_dram[:, blk * YB:(blk + 1) * YB])
                    y1t.append(y1blk)
                    ybuf = yb.tile([128, YB], BF16, tag="yb")
                    nc.scalar.activation(
                        ybuf[:], y_raw0[:, blk * YB:(blk + 1) * YB],
                        AF.Silu, scale=ab[:, 0, 0:1], bias=ab[:, 0, 1:2])
                    out_ap = _ap_raw(
                        out_d[:], blk * YB,
                        [(NS, 128), (1, YB)])
                    nc.sync.dma_start(out_ap, ybuf[:])
                    ybuf1 = yb.tile([128, YB], BF16, tag="yb")
                    nc.scalar.activation(
                        ybuf1[:], y1blk[:],
                        AF.Silu, scale=ab[:, 1, 0:1], bias=ab[:, 1, 1:2])
                    out_ap1 = _ap_raw(
                        out_d[:], 128 * NS + blk * YB,
                        [(NS, 128), (1, YB)])
                    nc.sync.dma_start(out_ap1, ybuf1[:])

    nc.compile()
    return nc


def prep_inputs(x, pw, pb, cw, gamma, beta):
    x = np.asarray(x, np.float32)
    pw = np.asarray(pw, np.float32)
    pb = np.asarray(pb, np.float32)
    cw = np.asarray(cw, np.float32)
    gamma = np.asarray(gamma, np.float32)
    beta = np.asarray(beta, np.float32)

    pwt = np.ascontiguousarray(
        pw.reshape(2 * N, INC, 9).transpose(1, 2, 0))      # (128, 9, 18)

    angles = np.linspace(0.0, 2.0 * math.pi, N + 1, dtype=np.float64)[:-1]
    pn = np.concatenate([np.cos(angles), np.sin(angles)]).astype(np.float32)
    p_idx = np.arange(128)
    t_idx = np.arange(32)
    hh = (2 * t_idx[None, :] + (p_idx[:, None] >= 64)).astype(np.float32)
    ww = np.broadcast_to((p_idx % 64).astype(np.float32)[:, None], (128, 32))
    base4 = np.zeros((128, 32, 2 * N), np.float32)
    base4[:, :, 0:N] = hh[:, :, None] + (pn[0:N] + pb[0:N])[None, None, :] + 4.0
    base4[:, :, N:] = ww[:, :, None] + (pn[N:] + pb[N:])[None, None, :] + 4.0

    cwt = np.ascontiguousarray(cw[:, :, 0, 0].T)           # (128, 256)
    gb = np.concatenate([gamma, beta])[None, :]            # (1, 512)
    id128 = np.eye(128, dtype=np.float32).astype(ml_dtypes.bfloat16)

    in_maps = []
    for b in range(B):
        xb = x[b].reshape(INC, S)
        xpad = np.zeros((INC, 66, 66), ml_dtypes.bfloat16)
        xpad[:, 1:65, 1:65] = x[b]
        xT = np.ascontiguousarray(xb.T).astype(ml_dtypes.bfloat16)  # (4096, 128)
        x2 = np.zeros((S, 2 * INC), ml_dtypes.bfloat16)
        x2[:, 0:INC] = xT
        x2[:S - 64, INC:] = xT[64:]
        in_maps.append(dict(
            xpad=np.ascontiguousarray(xpad.reshape(INC, 66 * 66)), x2=x2,
            pwt=pwt, base4=base4, cwt=cwt, gb=gb,
            id18=np.eye(18, dtype=np.float32), id128=id128,
            id128f=np.eye(128, dtype=np.float32)))
    return in_maps


_NC_CACHE = {}


def kernel(x, pw, pb, cw, gamma, beta):
    import os
    if "nc" not in _NC_CACHE:
        _NC_CACHE["nc"] = build(
            new_idx=os.environ.get("NEWIDX", "1") == "1",
            new_w=os.environ.get("NEWW", "1") == "1")
    nc = _NC_CACHE["nc"]
    in_maps = prep_inputs(x, pw, pb, cw, gamma, beta)
    res = run_bass_kernel_spmd(nc, in_maps, core_ids=list(range(N_CORES)))
    out = np.stack([
        np.asarray(res.results[b]["out"]).astype(np.float32).reshape(
            OUTC, H, W * N)
        for b in range(B)])
    return out

